# revision 30
# baseline (speedup 1.0000x reference)
"""Trainium2 Bass kernel for nn_MetricsLoss (contrastive + per-group metric losses).

Self-contained: accepts FULL inputs, shards rows across 8 NeuronCores
(data-parallel over contiguous group blocks), runs one SPMD Bass/Tile
program per core, and reduces the tiny per-group partial sums on host.

Per-core outputs:
  d_out     [65536]    f32  -- cosine-distance rows (graded output)
  stats_out [512, 16]  f32  -- per-group partial sums:
     0 sum_v, 1 sum_v2, 2 sum_d, 3 sum_d2, 4 sum_vd,
     5 cnt, 6 viol_sum, 7 neigh_sum, 8 sum_d_lab0, 9 n_lab0
"""

import numpy as np

# Problem constants (hardcoded per contract).
N, D, S = 524288, 128, 128
NCORES = 8
ROWS = N // NCORES            # 65536 rows per core
GC = ROWS // S                # 512 groups per core
GB = 128                      # groups per batch (= SBUF partitions)
NB = GC // GB                 # 4 batches
MACRO_G = 16                  # groups per z DMA macro-tile (1 MiB per tensor)
NMACRO = GB // MACRO_G        # 8 macro tiles per batch
PH1_DVE = 0                   # groups per macro-tile reduced on DVE (rest GPSIMD)
AQ = 16                       # "a" rows per pairwise chunk
NQ = S // AQ                  # 8 chunks
KM = 0.02                     # K_MARGIN
MARGIN = 2.0
EPS = 1e-6
NSTAT = 16

_CACHE = {}
TRACE = False          # set True (e.g. from test.py) to capture an NTFF trace
LAST_RES = None        # BassKernelResults of the most recent run


def build_bass(ph1_dve=PH1_DVE):
    """Build the per-core Bass/Tile program (same NEFF on all 8 cores)."""
    from contextlib import ExitStack

    import concourse.bass as bass
    import concourse.bacc as bacc
    import concourse.mybir as mybir
    import concourse.tile as tile

    op = mybir.AluOpType
    dt = mybir.dt
    AF = mybir.ActivationFunctionType
    AX = mybir.AxisListType
    F32, F16, I16, I32 = dt.float32, dt.float16, dt.int16, dt.int32

    nc = bacc.Bacc()
    zr_d = nc.dram_tensor("z_r", [ROWS, D], F32, kind="ExternalInput").ap()
    zv_d = nc.dram_tensor("z_v", [ROWS, D], F32, kind="ExternalInput").ap()
    vl_d = nc.dram_tensor("var_lens", [ROWS], I32, kind="ExternalInput").ap()
    lb_d = nc.dram_tensor("labels", [ROWS], I32, kind="ExternalInput").ap()
    do_d = nc.dram_tensor("d_out", [ROWS], F32, kind="ExternalOutput").ap()
    st_d = nc.dram_tensor("stats_out", [GC, NSTAT], F32, kind="ExternalOutput").ap()

    zr3 = zr_d.rearrange("(g s) d -> g s d", s=S)     # [512,128,128]
    zv3 = zv_d.rearrange("(g s) d -> g s d", s=S)
    vl2 = vl_d.rearrange("(g s) -> g s", s=S)         # [512,128]
    lb2 = lb_d.rearrange("(g s) -> g s", s=S)
    do2 = do_d.rearrange("(g s) -> g s", s=S)

    def dup_a_ap(dup128x256, q):
        # value-per-a duplicated pairs -> [(p),(2,AQ)@q,(0,64),(1,2)]
        a3 = dup128x256.rearrange("p (a two) -> p a two", two=2)
        a3 = a3[:, q * AQ:(q + 1) * AQ]
        a4 = a3.broadcast_to([128, AQ, 2, 64])        # (p, a, two, b64)
        return a4.transpose([0, 1, 3, 2])             # (p, a, b64, two)

    def b_side_ap(t128):
        # per-b values broadcast over a -> [(p),(0,AQ),(2,64),(1,2)]
        b3 = t128.rearrange("p (b64 two) -> p b64 two", two=2)
        b4 = b3.broadcast_to([128, 64, 2, AQ])        # (p, b64, two, a)
        return b4.transpose([0, 3, 1, 2])             # (p, a, b64, two)

    def q_view(scr):
        return scr.rearrange("p (a b64 two) -> p a b64 two", a=AQ, two=2)

    with tile.TileContext(nc) as tc, ExitStack() as ctx:
        def pool(name, bufs, **kw):
            return ctx.enter_context(tc.tile_pool(name=name, bufs=bufs, **kw))

        consts = pool("consts", 1)
        zpool = pool("z", 2)
        dpool = pool("dcols", 2 * NB * NMACRO)  # unique slots: no release waits
        gpool = pool("gscr", 4)
        tpool = pool("tiles", 2)
        kpool = pool("keep", NB)     # tiles consumed by the epilogue
        ppool = pool("pair", 2)
        spool = pool("small", 3)
        psum = pool("psum", 3, space="PSUM")

        # ---- one-time constants ----
        pos16 = consts.tile([128, S], I16)
        nc.gpsimd.iota(pos16[:], pattern=[[1, S]], base=0, channel_multiplier=0)
        ones_t = consts.tile([128, S], F32)
        nc.vector.memset(ones_t[:], 1.0)
        ident = consts.tile([128, S], F32)
        nc.gpsimd.affine_select(
            ident[:], ones_t[:], pattern=[[1, S]],
            compare_op=op.is_equal, fill=0.0, base=0, channel_multiplier=-1,
        )

        d16_list, rank_list, stats_list = [], [], []

        for b in range(NB):
            g0 = b * GB
            # ================= phase 1: d = 1 - sum(z_r*z_v) =================
            dps = psum.tile([128, S], F32, tag="dps")        # transposed [g, s]
            for m in range(NMACRO):
                gm = g0 + m * MACRO_G
                zr_t = zpool.tile([128, MACRO_G * D], F32, tag="zr")
                zv_t = zpool.tile([128, MACRO_G * D], F32, tag="zv")
                zr_v = zr_t[:].rearrange("p (a d) -> p a d", a=MACRO_G)
                zv_v = zv_t[:].rearrange("p (a d) -> p a d", a=MACRO_G)
                nc.sync.dma_start(zr_v, zr3[gm:gm + MACRO_G].transpose([1, 0, 2]))
                nc.sync.dma_start(zv_v, zv3[gm:gm + MACRO_G].transpose([1, 0, 2]))
                if m % 2 == 0:
                    # dsg/gsum span TWO macro-tiles (32 groups) so the
                    # transpose matmul output lands 32-aligned in PSUM.
                    dsg_m = dpool.tile([128, 2 * MACRO_G], F32, tag="dsg")
                    gsum_m = dpool.tile([128, 2 * MACRO_G], F32, tag="gsum")
                c_off = (m % 2) * MACRO_G
                # gates: absorb the cross-engine DMA waits on the Pool clock
                # (Pool engine instructions have a single ISA sync-wait slot,
                # so each gate may carry at most one DMA-queue wait).
                gate = gpool.tile([128, 1], F32, tag="gate")
                nc.gpsimd.tensor_copy(gate[:], zr_t[:, 0:1])
                gate2 = gpool.tile([128, 1], F32, tag="gate2")
                nc.gpsimd.tensor_copy(gate2[:], zv_t[:, 0:1])
                for a in range(ph1_dve):
                    col = c_off + a
                    pscr = gpool.tile([128, D], F32, tag="pscr")
                    nc.vector.scalar_tensor_tensor(
                        out=pscr[:], in0=zr_v[:, a], scalar=1.0,
                        in1=zv_v[:, a], op0=op.mult, op1=op.mult,
                        accum_out=gsum_m[:, col:col + 1],
                    )
                # GPSIMD multiplies the remaining groups; one DVE reduce
                # over d folds them into gsum (TensorScalarPtr is illegal
                # on Pool in this compiler).
                ng = MACRO_G - ph1_dve
                if ng:
                    prod = gpool.tile([128, ng * D], F32, tag="prod")
                    prod3 = prod[:].rearrange("p (a d) -> p a d", a=ng)
                    nc.gpsimd.tensor_tensor(
                        prod3, zr_v[:, ph1_dve:], zv_v[:, ph1_dve:], op.mult)
                    nc.vector.tensor_reduce(
                        gsum_m[:, c_off + ph1_dve:c_off + MACRO_G], prod3,
                        axis=AX.X, op=op.add)
                # d = 1 - sum  (Identity(scale*x+bias))
                nc.scalar.activation(
                    dsg_m[:, c_off:c_off + MACRO_G],
                    gsum_m[:, c_off:c_off + MACRO_G],
                    AF.Identity, bias=1.0, scale=-1.0,
                )
                if m % 2 == 1:
                    # transpose the pair's 32 columns into PSUM rows [32, 128]:
                    # out = dsg_m.T @ I  (regular matmul; transpose-mode would
                    # require PSUM partition 0)
                    p0 = (m - 1) * MACRO_G
                    nc.tensor.matmul(
                        dps[p0:p0 + 2 * MACRO_G, :], dsg_m[:], ident[:],
                        start=True, stop=True, tile_position=(0, p0))
            d32 = tpool.tile([128, S], F32, tag="d32")
            nc.vector.tensor_copy(d32[:], dps[:])
            nc.sync.dma_start(do2[g0:g0 + GB, :], d32[:])
            d16 = kpool.tile([128, S], F16, tag="d16")
            nc.vector.tensor_copy(d16[:], dps[:])
            d16_list.append(d16)

            # ---- per-batch small tiles ----
            vli = tpool.tile([128, S], I32, tag="vli")
            nc.sync.dma_start(vli[:], vl2[g0:g0 + GB, :])
            lbi = tpool.tile([128, S], I32, tag="lbi")
            nc.sync.dma_start(lbi[:], lb2[g0:g0 + GB, :])

            v32 = tpool.tile([128, S], F32, tag="v32")
            nc.vector.tensor_copy(v32[:], vli[:])
            v16i = tpool.tile([128, S], I16, tag="v16i")
            nc.vector.tensor_copy(v16i[:], vli[:])
            lb32 = tpool.tile([128, S], F32, tag="lb32")
            nc.vector.tensor_copy(lb32[:], lbi[:])
            kk16 = tpool.tile([128, S], I16, tag="kk16")
            nc.vector.scalar_tensor_tensor(
                out=kk16[:], in0=v16i[:], scalar=float(S), in1=pos16[:],
                op0=op.mult, op1=op.add,
            )

            # duplicated-pair tiles for the "a"-side operands
            def make_dup(src128, dtype, tag, bias=0.0):
                dup = tpool.tile([128, 2 * S], dtype, tag=tag)
                dst = dup[:].rearrange("p (a two) -> p a two", two=2)
                src = src128.rearrange("p (a one) -> p a one", one=1)
                src = src.broadcast_to([128, S, 2])
                if bias:
                    nc.vector.tensor_scalar_add(dst, src, bias)
                else:
                    nc.vector.tensor_copy(dst, src)
                return dup

            dkdup = make_dup(d32[:], F16, "dkdup", bias=KM)   # d_a + K
            vdup = make_dup(v16i[:], I16, "vdup")
            kkdup = make_dup(kk16[:], I16, "kkdup")

            # ================= stats =================
            stats = kpool.tile([128, NSTAT], F32, tag="stats")
            stats_list.append(stats)
            nc.vector.memset(stats[:], 0.0)
            nc.vector.tensor_reduce(stats[:, 0:1], v32[:], axis=AX.X, op=op.add)
            s1 = gpool.tile([128, S], F32, tag="sscr")
            nc.vector.scalar_tensor_tensor(
                out=s1[:], in0=v32[:], scalar=1.0, in1=v32[:],
                op0=op.mult, op1=op.mult, accum_out=stats[:, 1:2])
            nc.vector.tensor_reduce(stats[:, 2:3], d32[:], axis=AX.X, op=op.add)
            s2 = gpool.tile([128, S], F32, tag="sscr")
            nc.vector.scalar_tensor_tensor(
                out=s2[:], in0=d32[:], scalar=1.0, in1=d32[:],
                op0=op.mult, op1=op.mult, accum_out=stats[:, 3:4])
            s3 = gpool.tile([128, S], F32, tag="sscr")
            nc.vector.scalar_tensor_tensor(
                out=s3[:], in0=v32[:], scalar=1.0, in1=d32[:],
                op0=op.mult, op1=op.mult, accum_out=stats[:, 4:5])
            s4 = gpool.tile([128, S], F32, tag="sscr")
            nc.vector.scalar_tensor_tensor(
                out=s4[:], in0=lb32[:], scalar=0.0, in1=d32[:],
                op0=op.is_equal, op1=op.mult, accum_out=stats[:, 8:9])
            s5 = gpool.tile([128, S], F32, tag="sscr")
            nc.vector.tensor_scalar(
                out=s5[:], in0=lb32[:], scalar1=0.0, scalar2=0.0,
                op0=op.is_equal, op1=op.add, accum_out=stats[:, 9:10])

            # ================= pairwise quarters =================
            violq = spool.tile([128, NQ], F32, tag="violq")
            cntq = spool.tile([128, NQ], F32, tag="cntq")
            rank = kpool.tile([128, S], I16, tag="rank")
            rank_list.append(rank)

            # pass A: rank[a] = #{b: kk_b < kk_a} via compare + tree-reduce
            for q in range(NQ):
                kkq = ppool.tile([128, AQ * S], I16, tag="KK")
                nc.vector.tensor_tensor(
                    q_view(kkq[:]), dup_a_ap(kkdup[:], q), b_side_ap(kk16[:]),
                    op.is_gt)
                # tree-reduce over b (7 levels) -> rank[:, q*AQ:(q+1)*AQ]
                cur = kkq[:].rearrange("p (a c) -> p a c", a=AQ)
                width = S
                for lvl in range(7):
                    width //= 2
                    if width > 1:
                        nxt_t = ppool.tile([128, AQ * width], I16, tag=f"tr{lvl}")
                        nxt = nxt_t[:].rearrange("p (a c) -> p a c", a=AQ)
                    else:
                        nxt = rank[:, q * AQ:(q + 1) * AQ].rearrange(
                            "p (a c) -> p a c", c=1)
                    nc.vector.tensor_tensor(
                        nxt, cur[:, :, 0:width], cur[:, :, width:2 * width],
                        op.add)
                    cur = nxt

            # pass B: viol/cnt/neigh
            for q in range(NQ):
                tq = ppool.tile([128, AQ * S], F16, tag="T")
                nc.vector.tensor_tensor(
                    q_view(tq[:]), dup_a_ap(dkdup[:], q), b_side_ap(d16[:]),
                    op.subtract)
                mq = ppool.tile([128, AQ * S], F16, tag="M")
                nc.vector.tensor_tensor(
                    q_view(mq[:]), dup_a_ap(vdup[:], q), b_side_ap(v16i[:]),
                    op.is_lt)
                mt = ppool.tile([128, AQ * S], F16, tag="MT")
                nc.vector.tensor_tensor(mt[:], mq[:], tq[:], op.mult)
                vs_ = ppool.tile([128, AQ * S], F16, tag="ascr")
                nc.scalar.activation(
                    vs_[:], mt[:], AF.Relu, accum_out=violq[:, q:q + 1])
                cs_ = ppool.tile([128, AQ * S], F16, tag="ascr")
                nc.scalar.activation(
                    cs_[:], mq[:], AF.Copy, accum_out=cntq[:, q:q + 1])

            nc.vector.tensor_reduce(stats[:, 6:7], violq[:], axis=AX.X, op=op.add)
            nc.vector.tensor_reduce(stats[:, 5:6], cntq[:], axis=AX.X, op=op.add)

        # ===== epilogue: neighbour term via per-partition rank-scatter =====
        # (library switch handled automatically by Bacc.insert_library_loads)
        for b in range(NB):
            g0 = b * GB
            stats = stats_list[b]
            dsort = kpool.tile([128, S], F16, tag="dsort")
            nc.gpsimd.local_scatter(
                dsort[:], d16_list[b][:], rank_list[b][:],
                channels=128, num_elems=S, num_idxs=S)
            ntmp = spool.tile([128, S], F16, tag="ntmp")
            nc.vector.scalar_tensor_tensor(
                out=ntmp[:, 0:S - 1], in0=dsort[:, 0:S - 1], scalar=KM,
                in1=dsort[:, 1:S], op0=op.add, op1=op.subtract)
            ntmp2 = spool.tile([128, S], F16, tag="ntmp2")
            nc.scalar.activation(
                ntmp2[:, 0:S - 1], ntmp[:, 0:S - 1], AF.Relu,
                accum_out=stats[:, 7:8])
            nc.sync.dma_start(st_d[g0:g0 + GB, :], stats[:])

    return nc


def _get_nc():
    if "nc" not in _CACHE:
        nc = build_bass()
        nc.finalize()   # Bacc: wait-splitting, library loads, ISA packing
        _CACHE["nc"] = nc
    return _CACHE["nc"]


def _host_finalize(d, stats):
    st = stats.astype(np.float64)
    sv, svv, sd, sdd, svd = st[:, 0], st[:, 1], st[:, 2], st[:, 3], st[:, 4]
    cnt, viol, nsum, sdl0, nl0 = st[:, 5], st[:, 6], st[:, 7], st[:, 8], st[:, 9]
    Sf = float(S)

    Svv = svv - sv * sv / Sf
    Sdd = sdd - sd * sd / Sf
    Svd = svd - sv * sd / Sf
    vs = np.sqrt(np.maximum(Svv / (Sf - 1.0), 0.0))
    ds = np.sqrt(np.maximum(Sdd / (Sf - 1.0), 0.0))
    corr = (Svv / (vs + EPS) ** 2 + Sdd / (ds + EPS) ** 2
            - 2.0 * Svd / ((vs + EPS) * (ds + EPS))) / Sf
    corr = np.where((vs > 0) & (ds > 0), corr, 0.0)

    neigh = nsum / (Sf - 1.0)
    rank = np.where(cnt > 0, viol / np.maximum(cnt, 1.0), 0.0)
    l_pcc = np.float32(np.mean(corr + neigh + rank))

    nb = nl0.sum()
    npos = float(N) - nb
    sum_b = sdl0.sum()
    sum_p = sd.sum() - sum_b
    mean_b = sum_b / max(nb, 1.0)
    mean_p = sum_p / max(npos, 1.0)
    l_cdd = max(MARGIN + mean_b - mean_p, 0.0) if (nb > 0 and npos > 0) else 0.0
    return np.float32(l_cdd), l_pcc


def kernel(z_r, z_v, labels, groups, var_lens):
    from concourse.bass_utils import run_bass_kernel_spmd

    z_r = np.asarray(z_r, dtype=np.float32)
    z_v = np.asarray(z_v, dtype=np.float32)
    labels = np.asarray(labels, dtype=np.int32)
    var_lens = np.asarray(var_lens, dtype=np.int32)

    nc = _get_nc()
    in_maps = []
    for c in range(NCORES):
        sl = slice(c * ROWS, (c + 1) * ROWS)
        in_maps.append({
            "z_r": np.ascontiguousarray(z_r[sl]),
            "z_v": np.ascontiguousarray(z_v[sl]),
            "var_lens": np.ascontiguousarray(var_lens[sl]),
            "labels": np.ascontiguousarray(labels[sl]),
        })
    res = run_bass_kernel_spmd(
        nc, in_maps, core_ids=list(range(NCORES)), trace=TRACE)
    global LAST_RES
    LAST_RES = res
    d = np.concatenate([r["d_out"] for r in res.results]).astype(np.float32)
    stats = np.concatenate([r["stats_out"] for r in res.results], axis=0)
    l_cdd, l_pcc = _host_finalize(d, stats)
    return l_cdd, l_pcc, d


# revision 31
# speedup vs baseline: 1.0493x; 1.0493x over previous
"""Trainium2 Bass kernel for nn_MetricsLoss (contrastive + per-group metric losses).

Self-contained: accepts FULL inputs, shards rows across 8 NeuronCores
(data-parallel over contiguous group blocks), runs one SPMD Bass/Tile
program per core, and reduces the tiny per-group partial sums on host.

Per-core outputs:
  d_out     [65536]    f32  -- cosine-distance rows (graded output)
  stats_out [512, 16]  f32  -- per-group partial sums:
     0 sum_v, 1 sum_v2, 2 sum_d, 3 sum_d2, 4 sum_vd,
     5 cnt, 6 viol_sum, 7 neigh_sum, 8 sum_d_lab0, 9 n_lab0
"""

import numpy as np

# Problem constants (hardcoded per contract).
N, D, S = 524288, 128, 128
NCORES = 8
ROWS = N // NCORES            # 65536 rows per core
GC = ROWS // S                # 512 groups per core
GB = 128                      # groups per batch (= SBUF partitions)
NB = GC // GB                 # 4 batches
MACRO_G = 16                  # groups per z DMA macro-tile (1 MiB per tensor)
NMACRO = GB // MACRO_G        # 8 macro tiles per batch
PH1_DVE = 0                   # groups per macro-tile reduced on DVE (rest GPSIMD)
AQ = 16                       # "a" rows per pairwise chunk
NQ = S // AQ                  # 8 chunks
KM = 0.02                     # K_MARGIN
MARGIN = 2.0
EPS = 1e-6
NSTAT = 16

_CACHE = {}
TRACE = False          # set True (e.g. from test.py) to capture an NTFF trace
LAST_RES = None        # BassKernelResults of the most recent run


def build_bass(ph1_dve=PH1_DVE):
    """Build the per-core Bass/Tile program (same NEFF on all 8 cores)."""
    from contextlib import ExitStack

    import concourse.bass as bass
    import concourse.bacc as bacc
    import concourse.mybir as mybir
    import concourse.tile as tile

    op = mybir.AluOpType
    dt = mybir.dt
    AF = mybir.ActivationFunctionType
    AX = mybir.AxisListType
    F32, F16, I16, I32 = dt.float32, dt.float16, dt.int16, dt.int32

    nc = bacc.Bacc()
    zr_d = nc.dram_tensor("z_r", [ROWS, D], F32, kind="ExternalInput").ap()
    zv_d = nc.dram_tensor("z_v", [ROWS, D], F32, kind="ExternalInput").ap()
    vl_d = nc.dram_tensor("var_lens", [ROWS], I32, kind="ExternalInput").ap()
    lb_d = nc.dram_tensor("labels", [ROWS], I32, kind="ExternalInput").ap()
    do_d = nc.dram_tensor("d_out", [ROWS], F32, kind="ExternalOutput").ap()
    st_d = nc.dram_tensor("stats_out", [GC, NSTAT], F32, kind="ExternalOutput").ap()

    zr3 = zr_d.rearrange("(g s) d -> g s d", s=S)     # [512,128,128]
    zv3 = zv_d.rearrange("(g s) d -> g s d", s=S)
    vl2 = vl_d.rearrange("(g s) -> g s", s=S)         # [512,128]
    lb2 = lb_d.rearrange("(g s) -> g s", s=S)
    do2 = do_d.rearrange("(g s) -> g s", s=S)

    def dup_a_ap(dup128x256, q):
        # value-per-a duplicated pairs -> [(p),(2,AQ)@q,(0,64),(1,2)]
        a3 = dup128x256.rearrange("p (a two) -> p a two", two=2)
        a3 = a3[:, q * AQ:(q + 1) * AQ]
        a4 = a3.broadcast_to([128, AQ, 2, 64])        # (p, a, two, b64)
        return a4.transpose([0, 1, 3, 2])             # (p, a, b64, two)

    def b_side_ap(t128):
        # per-b values broadcast over a -> [(p),(0,AQ),(2,64),(1,2)]
        b3 = t128.rearrange("p (b64 two) -> p b64 two", two=2)
        b4 = b3.broadcast_to([128, 64, 2, AQ])        # (p, b64, two, a)
        return b4.transpose([0, 3, 1, 2])             # (p, a, b64, two)

    def q_view(scr):
        return scr.rearrange("p (a b64 two) -> p a b64 two", a=AQ, two=2)

    with tile.TileContext(nc) as tc, ExitStack() as ctx:
        def pool(name, bufs, **kw):
            return ctx.enter_context(tc.tile_pool(name=name, bufs=bufs, **kw))

        consts = pool("consts", 1)
        zpool = pool("z", 2)
        dpool = pool("dcols", 2 * NB * NMACRO)  # unique slots: no release waits
        gpool = pool("gscr", 4)
        tpool = pool("tiles", 2)
        kpool = pool("keep", NB)     # tiles consumed by the epilogue
        ppool = pool("pair", 2)
        spool = pool("small", 3)
        psum = pool("psum", 3, space="PSUM")

        # ---- one-time constants ----
        pos16 = consts.tile([128, S], I16)
        nc.gpsimd.iota(pos16[:], pattern=[[1, S]], base=0, channel_multiplier=0)
        ones_t = consts.tile([128, S], F32)
        nc.vector.memset(ones_t[:], 1.0)
        ident = consts.tile([128, S], F32)
        nc.gpsimd.affine_select(
            ident[:], ones_t[:], pattern=[[1, S]],
            compare_op=op.is_equal, fill=0.0, base=0, channel_multiplier=-1,
        )

        d16_list, rank_list, stats_list = [], [], []

        for b in range(NB):
            g0 = b * GB
            # ================= phase 1: d = 1 - sum(z_r*z_v) =================
            dps = psum.tile([128, S], F32, tag="dps")        # transposed [g, s]
            for m in range(NMACRO):
                gm = g0 + m * MACRO_G
                zr_t = zpool.tile([128, MACRO_G * D], F32, tag="zr")
                zv_t = zpool.tile([128, MACRO_G * D], F32, tag="zv")
                zr_v = zr_t[:].rearrange("p (a d) -> p a d", a=MACRO_G)
                zv_v = zv_t[:].rearrange("p (a d) -> p a d", a=MACRO_G)
                nc.sync.dma_start(zr_v, zr3[gm:gm + MACRO_G].transpose([1, 0, 2]))
                nc.sync.dma_start(zv_v, zv3[gm:gm + MACRO_G].transpose([1, 0, 2]))
                if m % 2 == 0:
                    # dsg/gsum span TWO macro-tiles (32 groups) so the
                    # transpose matmul output lands 32-aligned in PSUM.
                    dsg_m = dpool.tile([128, 2 * MACRO_G], F32, tag="dsg")
                    gsum_m = dpool.tile([128, 2 * MACRO_G], F32, tag="gsum")
                c_off = (m % 2) * MACRO_G
                # gates: absorb the cross-engine DMA waits on the Pool clock
                # (Pool engine instructions have a single ISA sync-wait slot,
                # so each gate may carry at most one DMA-queue wait).
                gate = gpool.tile([128, 1], F32, tag="gate")
                nc.gpsimd.tensor_copy(gate[:], zr_t[:, 0:1])
                gate2 = gpool.tile([128, 1], F32, tag="gate2")
                nc.gpsimd.tensor_copy(gate2[:], zv_t[:, 0:1])
                for a in range(ph1_dve):
                    col = c_off + a
                    pscr = gpool.tile([128, D], F32, tag="pscr")
                    nc.vector.scalar_tensor_tensor(
                        out=pscr[:], in0=zr_v[:, a], scalar=1.0,
                        in1=zv_v[:, a], op0=op.mult, op1=op.mult,
                        accum_out=gsum_m[:, col:col + 1],
                    )
                # GPSIMD multiplies the remaining groups; one DVE reduce
                # over d folds them into gsum (TensorScalarPtr is illegal
                # on Pool in this compiler).
                ng = MACRO_G - ph1_dve
                if ng:
                    prod = gpool.tile([128, ng * D], F32, tag="prod")
                    prod3 = prod[:].rearrange("p (a d) -> p a d", a=ng)
                    nc.gpsimd.tensor_tensor(
                        prod3, zr_v[:, ph1_dve:], zv_v[:, ph1_dve:], op.mult)
                    # fold the two d-halves in place on Pool so the DVE
                    # reduce only covers 64 elements per group
                    nc.gpsimd.tensor_tensor(
                        prod3[:, :, 0:D // 2], prod3[:, :, 0:D // 2],
                        prod3[:, :, D // 2:D], op.add)
                    nc.vector.tensor_reduce(
                        gsum_m[:, c_off + ph1_dve:c_off + MACRO_G],
                        prod3[:, :, 0:D // 2], axis=AX.X, op=op.add)
                # d = 1 - sum  (Identity(scale*x+bias))
                nc.scalar.activation(
                    dsg_m[:, c_off:c_off + MACRO_G],
                    gsum_m[:, c_off:c_off + MACRO_G],
                    AF.Identity, bias=1.0, scale=-1.0,
                )
                if m % 2 == 1:
                    # transpose the pair's 32 columns into PSUM rows [32, 128]:
                    # out = dsg_m.T @ I  (regular matmul; transpose-mode would
                    # require PSUM partition 0)
                    p0 = (m - 1) * MACRO_G
                    nc.tensor.matmul(
                        dps[p0:p0 + 2 * MACRO_G, :], dsg_m[:], ident[:],
                        start=True, stop=True, tile_position=(0, p0))
            d32 = tpool.tile([128, S], F32, tag="d32")
            nc.vector.tensor_copy(d32[:], dps[:])
            nc.sync.dma_start(do2[g0:g0 + GB, :], d32[:])
            d16 = kpool.tile([128, S], F16, tag="d16")
            nc.vector.tensor_copy(d16[:], dps[:])
            d16_list.append(d16)

            # ---- per-batch small tiles ----
            vli = tpool.tile([128, S], I32, tag="vli")
            nc.sync.dma_start(vli[:], vl2[g0:g0 + GB, :])
            lbi = tpool.tile([128, S], I32, tag="lbi")
            nc.sync.dma_start(lbi[:], lb2[g0:g0 + GB, :])

            v32 = tpool.tile([128, S], F32, tag="v32")
            nc.vector.tensor_copy(v32[:], vli[:])
            v16i = tpool.tile([128, S], I16, tag="v16i")
            nc.vector.tensor_copy(v16i[:], vli[:])
            lb32 = tpool.tile([128, S], F32, tag="lb32")
            nc.vector.tensor_copy(lb32[:], lbi[:])
            kk16 = tpool.tile([128, S], I16, tag="kk16")
            nc.vector.scalar_tensor_tensor(
                out=kk16[:], in0=v16i[:], scalar=float(S), in1=pos16[:],
                op0=op.mult, op1=op.add,
            )

            # duplicated-pair tiles for the "a"-side operands
            def make_dup(src128, dtype, tag, bias=0.0):
                dup = tpool.tile([128, 2 * S], dtype, tag=tag)
                dst = dup[:].rearrange("p (a two) -> p a two", two=2)
                src = src128.rearrange("p (a one) -> p a one", one=1)
                src = src.broadcast_to([128, S, 2])
                if bias:
                    nc.vector.tensor_scalar_add(dst, src, bias)
                else:
                    nc.vector.tensor_copy(dst, src)
                return dup

            dkdup = make_dup(d32[:], F16, "dkdup", bias=KM)   # d_a + K
            vdup = make_dup(v16i[:], I16, "vdup")
            kkdup = make_dup(kk16[:], I16, "kkdup")

            # ================= stats =================
            stats = kpool.tile([128, NSTAT], F32, tag="stats")
            stats_list.append(stats)
            nc.vector.memset(stats[:], 0.0)
            nc.vector.tensor_reduce(stats[:, 0:1], v32[:], axis=AX.X, op=op.add)
            s1 = gpool.tile([128, S], F32, tag="sscr")
            nc.vector.scalar_tensor_tensor(
                out=s1[:], in0=v32[:], scalar=1.0, in1=v32[:],
                op0=op.mult, op1=op.mult, accum_out=stats[:, 1:2])
            nc.vector.tensor_reduce(stats[:, 2:3], d32[:], axis=AX.X, op=op.add)
            s2 = gpool.tile([128, S], F32, tag="sscr")
            nc.vector.scalar_tensor_tensor(
                out=s2[:], in0=d32[:], scalar=1.0, in1=d32[:],
                op0=op.mult, op1=op.mult, accum_out=stats[:, 3:4])
            s3 = gpool.tile([128, S], F32, tag="sscr")
            nc.vector.scalar_tensor_tensor(
                out=s3[:], in0=v32[:], scalar=1.0, in1=d32[:],
                op0=op.mult, op1=op.mult, accum_out=stats[:, 4:5])
            s4 = gpool.tile([128, S], F32, tag="sscr")
            nc.vector.scalar_tensor_tensor(
                out=s4[:], in0=lb32[:], scalar=0.0, in1=d32[:],
                op0=op.is_equal, op1=op.mult, accum_out=stats[:, 8:9])
            s5 = gpool.tile([128, S], F32, tag="sscr")
            nc.vector.tensor_scalar(
                out=s5[:], in0=lb32[:], scalar1=0.0, scalar2=0.0,
                op0=op.is_equal, op1=op.add, accum_out=stats[:, 9:10])

            # ================= pairwise quarters =================
            violq = spool.tile([128, NQ], F32, tag="violq")
            cntq = spool.tile([128, NQ], F32, tag="cntq")
            rank = kpool.tile([128, S], I16, tag="rank")
            rank_list.append(rank)

            # pass A: rank[a] = #{b: kk_b < kk_a} via compare + tree-reduce
            for q in range(NQ):
                kkq = ppool.tile([128, AQ * S], I16, tag="KK")
                nc.vector.tensor_tensor(
                    q_view(kkq[:]), dup_a_ap(kkdup[:], q), b_side_ap(kk16[:]),
                    op.is_gt)
                # tree-reduce over b (7 levels) -> rank[:, q*AQ:(q+1)*AQ]
                cur = kkq[:].rearrange("p (a c) -> p a c", a=AQ)
                width = S
                for lvl in range(7):
                    width //= 2
                    if width > 1:
                        nxt_t = ppool.tile([128, AQ * width], I16, tag=f"tr{lvl}")
                        nxt = nxt_t[:].rearrange("p (a c) -> p a c", a=AQ)
                    else:
                        nxt = rank[:, q * AQ:(q + 1) * AQ].rearrange(
                            "p (a c) -> p a c", c=1)
                    nc.vector.tensor_tensor(
                        nxt, cur[:, :, 0:width], cur[:, :, width:2 * width],
                        op.add)
                    cur = nxt

            # pass B: viol/cnt/neigh
            for q in range(NQ):
                tq = ppool.tile([128, AQ * S], F16, tag="T")
                nc.vector.tensor_tensor(
                    q_view(tq[:]), dup_a_ap(dkdup[:], q), b_side_ap(d16[:]),
                    op.subtract)
                mq = ppool.tile([128, AQ * S], F16, tag="M")
                nc.vector.tensor_tensor(
                    q_view(mq[:]), dup_a_ap(vdup[:], q), b_side_ap(v16i[:]),
                    op.is_lt)
                mt = ppool.tile([128, AQ * S], F16, tag="MT")
                nc.vector.tensor_tensor(mt[:], mq[:], tq[:], op.mult)
                vs_ = ppool.tile([128, AQ * S], F16, tag="ascr")
                nc.scalar.activation(
                    vs_[:], mt[:], AF.Relu, accum_out=violq[:, q:q + 1])
                cs_ = ppool.tile([128, AQ * S], F16, tag="ascr")
                nc.scalar.activation(
                    cs_[:], mq[:], AF.Copy, accum_out=cntq[:, q:q + 1])

            nc.vector.tensor_reduce(stats[:, 6:7], violq[:], axis=AX.X, op=op.add)
            nc.vector.tensor_reduce(stats[:, 5:6], cntq[:], axis=AX.X, op=op.add)

        # ===== epilogue: neighbour term via per-partition rank-scatter =====
        # (library switch handled automatically by Bacc.insert_library_loads)
        for b in range(NB):
            g0 = b * GB
            stats = stats_list[b]
            dsort = kpool.tile([128, S], F16, tag="dsort")
            nc.gpsimd.local_scatter(
                dsort[:], d16_list[b][:], rank_list[b][:],
                channels=128, num_elems=S, num_idxs=S)
            ntmp = spool.tile([128, S], F16, tag="ntmp")
            nc.vector.scalar_tensor_tensor(
                out=ntmp[:, 0:S - 1], in0=dsort[:, 0:S - 1], scalar=KM,
                in1=dsort[:, 1:S], op0=op.add, op1=op.subtract)
            ntmp2 = spool.tile([128, S], F16, tag="ntmp2")
            nc.scalar.activation(
                ntmp2[:, 0:S - 1], ntmp[:, 0:S - 1], AF.Relu,
                accum_out=stats[:, 7:8])
            nc.sync.dma_start(st_d[g0:g0 + GB, :], stats[:])

    return nc


def _get_nc():
    if "nc" not in _CACHE:
        nc = build_bass()
        nc.finalize()   # Bacc: wait-splitting, library loads, ISA packing
        _CACHE["nc"] = nc
    return _CACHE["nc"]


def _host_finalize(d, stats):
    st = stats.astype(np.float64)
    sv, svv, sd, sdd, svd = st[:, 0], st[:, 1], st[:, 2], st[:, 3], st[:, 4]
    cnt, viol, nsum, sdl0, nl0 = st[:, 5], st[:, 6], st[:, 7], st[:, 8], st[:, 9]
    Sf = float(S)

    Svv = svv - sv * sv / Sf
    Sdd = sdd - sd * sd / Sf
    Svd = svd - sv * sd / Sf
    vs = np.sqrt(np.maximum(Svv / (Sf - 1.0), 0.0))
    ds = np.sqrt(np.maximum(Sdd / (Sf - 1.0), 0.0))
    corr = (Svv / (vs + EPS) ** 2 + Sdd / (ds + EPS) ** 2
            - 2.0 * Svd / ((vs + EPS) * (ds + EPS))) / Sf
    corr = np.where((vs > 0) & (ds > 0), corr, 0.0)

    neigh = nsum / (Sf - 1.0)
    rank = np.where(cnt > 0, viol / np.maximum(cnt, 1.0), 0.0)
    l_pcc = np.float32(np.mean(corr + neigh + rank))

    nb = nl0.sum()
    npos = float(N) - nb
    sum_b = sdl0.sum()
    sum_p = sd.sum() - sum_b
    mean_b = sum_b / max(nb, 1.0)
    mean_p = sum_p / max(npos, 1.0)
    l_cdd = max(MARGIN + mean_b - mean_p, 0.0) if (nb > 0 and npos > 0) else 0.0
    return np.float32(l_cdd), l_pcc


def kernel(z_r, z_v, labels, groups, var_lens):
    from concourse.bass_utils import run_bass_kernel_spmd

    z_r = np.asarray(z_r, dtype=np.float32)
    z_v = np.asarray(z_v, dtype=np.float32)
    labels = np.asarray(labels, dtype=np.int32)
    var_lens = np.asarray(var_lens, dtype=np.int32)

    nc = _get_nc()
    in_maps = []
    for c in range(NCORES):
        sl = slice(c * ROWS, (c + 1) * ROWS)
        in_maps.append({
            "z_r": np.ascontiguousarray(z_r[sl]),
            "z_v": np.ascontiguousarray(z_v[sl]),
            "var_lens": np.ascontiguousarray(var_lens[sl]),
            "labels": np.ascontiguousarray(labels[sl]),
        })
    res = run_bass_kernel_spmd(
        nc, in_maps, core_ids=list(range(NCORES)), trace=TRACE)
    global LAST_RES
    LAST_RES = res
    d = np.concatenate([r["d_out"] for r in res.results]).astype(np.float32)
    stats = np.concatenate([r["stats_out"] for r in res.results], axis=0)
    l_cdd, l_pcc = _host_finalize(d, stats)
    return l_cdd, l_pcc, d
